# revision 3
# baseline (speedup 1.0000x reference)
"""Trainium2 Bass kernel for the graph top-k pooling module (nn_Pool).

Math (reference):
    gI = g with diagonal forced to 1            [N, N], 0/1
    G2 = gI @ gI ; U = (G2 != 0)                [N, N]
    score = sum_heads sigmoid(h @ W.T + b)      [N]
    vals, idx = top_k(score, kk)                kk = N/2
    new_h = h[idx, :] * vals[:, None]
    un_g = ((U @ U) != 0)[idx, :][:, idx]       [kk, kk]
    out  = un_g / rowsum(un_g), new_h, idx

Key algebraic reduction: row/col selection commutes with the boolean
matmul, so only
    AT = U[idx, :].T = bin(gI^T @ gI[idx, :].T)     [N, kk]
    B  = U[:, idx]   = bin(gI @ gI[:, idx])          [N, kk]
    C  = bin(AT.T @ B)                               [kk, kk]
are needed: 1.375 TFLOP instead of 2.2 TFLOP.  All matmul operands are
exactly 0/1 so fp8e4m3 inputs with fp32 PSUM accumulation are exact
(counts <= 8192 << 2^24); binarize via min(x, 1).

Distribution (8 cores, 2 launches):
  L1: core i computes AT rows [1024*i : 1024*(i+1)]  (lhsT = gI[:, cols_i],
      rhs = gI[idx,:].T) and B rows [1024*i:1024*(i+1)] (lhsT = gI.T[:, cols_i],
      rhs = gI[:, idx]).  Host gathers AT, B.
  L2: core i computes C rows [512*i : 512*(i+1)] = AT[:, slice].T @ B
      as raw fp32 counts.  Host binarizes + row-normalizes (exact fp32 ops).

score/topk/new_h are replicated bit-exactly with the same eager jax ops
as the reference (tiny: 8M FLOP).
"""

import numpy as np

N = 8192
KK = 4096
NCORES = 8
MB = N // NCORES  # 1024: AT/B row-block per core in L1
MC = KK // NCORES  # 512: C row-block per core in L2

_CACHE = {}

# perf results of the last kernel() call: list of (name, exec_time_ns or None)
LAST_PERF = []


def _binarize_evict(nc, psum, sbuf):
    # counts >= 0 are exact integers in PSUM; min(x, 1) -> exact 0/1
    nc.vector.tensor_scalar_min(sbuf, psum, 1.0)


def _build_k1():
    """Launch-1 program: two [8192k x 1024m] @ [8192k x 4096n] fp8 matmuls
    with binarize-on-evict, outputs fp8."""
    import concourse.mybir as mybir
    import concourse.tile as tile
    from concourse import bacc
    from concourse.kernels.tile_matmul import matmul_tile_kernel

    f8 = mybir.dt.float8e4
    nc = bacc.Bacc("TRN2", target_bir_lowering=False, debug=False)
    lhsT_at = nc.dram_tensor("lhsT_at", [N, MB], f8, kind="ExternalInput")
    rhs_at = nc.dram_tensor("rhs_at", [N, KK], f8, kind="ExternalInput")
    lhsT_b = nc.dram_tensor("lhsT_b", [N, MB], f8, kind="ExternalInput")
    rhs_b = nc.dram_tensor("rhs_b", [N, KK], f8, kind="ExternalInput")
    out_at = nc.dram_tensor("out_at", [MB, KK], f8, kind="ExternalOutput")
    out_b = nc.dram_tensor("out_b", [MB, KK], f8, kind="ExternalOutput")
    with tile.TileContext(nc) as tc:
        matmul_tile_kernel(
            tc, lhsT_at[:], rhs_at[:], out_at[:], psum_evict_fn=_binarize_evict
        )
        matmul_tile_kernel(
            tc, lhsT_b[:], rhs_b[:], out_b[:], psum_evict_fn=_binarize_evict
        )
    nc.compile()
    return nc


def _build_k2():
    """Launch-2 program: [8192k x 512m] @ [8192k x 4096n] fp8 matmul,
    raw fp32 counts out."""
    import concourse.mybir as mybir
    import concourse.tile as tile
    from concourse import bacc
    from concourse.kernels.tile_matmul import matmul_tile_kernel

    f8 = mybir.dt.float8e4
    f32 = mybir.dt.float32
    nc = bacc.Bacc("TRN2", target_bir_lowering=False, debug=False)
    lhsT_c = nc.dram_tensor("lhsT_c", [N, MC], f8, kind="ExternalInput")
    rhs_c = nc.dram_tensor("rhs_c", [N, KK], f8, kind="ExternalInput")
    out_c = nc.dram_tensor("out_c", [MC, KK], f32, kind="ExternalOutput")
    with tile.TileContext(nc) as tc:
        matmul_tile_kernel(tc, lhsT_c[:], rhs_c[:], out_c[:])
    nc.compile()
    return nc


def _get(name, builder):
    if name not in _CACHE:
        _CACHE[name] = builder()
    return _CACHE[name]


def _run_spmd(nc, in_maps, core_ids):
    """run_bass_kernel_spmd, falling back to trace-disabled execution if the
    profiling path is unavailable in this environment."""
    import os
    from concourse.bass_utils import run_bass_kernel_spmd

    try:
        return run_bass_kernel_spmd(nc, in_maps, core_ids)
    except (ImportError, ModuleNotFoundError):
        old = os.environ.get("BASS_NEVER_TRACE")
        os.environ["BASS_NEVER_TRACE"] = "1"
        try:
            return run_bass_kernel_spmd(nc, in_maps, core_ids)
        finally:
            if old is None:
                os.environ.pop("BASS_NEVER_TRACE", None)
            else:
                os.environ["BASS_NEVER_TRACE"] = old


def kernel(g, h, W, b):
    import ml_dtypes
    import jax
    import jax.numpy as jnp

    global LAST_PERF
    LAST_PERF = []

    # ---- score / topk / new_h: bit-exact replication of the reference ----
    h_j = jnp.asarray(h)
    score = jnp.sum(jax.nn.sigmoid(h_j @ jnp.asarray(W).T + jnp.asarray(b)), axis=-1)
    vals, idx_j = jax.lax.top_k(score, KK)
    new_h = np.asarray(h_j[idx_j, :] * vals[:, None])
    idx = np.asarray(idx_j)

    # ---- build 0/1 fp8 operands on host ----
    f8 = ml_dtypes.float8_e4m3
    one8 = np.float32(1.0).astype(f8).view(np.uint8)  # bit pattern of 1.0
    g_np = np.asarray(g)
    gI = np.where(g_np != 0, one8, np.uint8(0))
    np.fill_diagonal(gI, one8)
    gIT = np.ascontiguousarray(gI.T)
    rhs_at = np.ascontiguousarray(gIT[:, idx]).view(f8)  # gI[idx,:].T  [N, KK]
    rhs_b = np.ascontiguousarray(gI[:, idx]).view(f8)  # gI[:, idx]   [N, KK]

    core_ids = list(range(NCORES))
    in_maps1 = []
    for i in core_ids:
        sl = slice(MB * i, MB * (i + 1))
        in_maps1.append(
            {
                "lhsT_at": np.ascontiguousarray(gI[:, sl]).view(f8),
                "rhs_at": rhs_at,
                "lhsT_b": np.ascontiguousarray(gIT[:, sl]).view(f8),
                "rhs_b": rhs_b,
            }
        )

    nc1 = _get("k1", _build_k1)
    res1 = _run_spmd(nc1, in_maps1, core_ids)
    LAST_PERF.append(("launch1", res1.exec_time_ns))

    AT = np.concatenate([r["out_at"] for r in res1.results], axis=0)  # [N, KK] fp8
    B = np.concatenate([r["out_b"] for r in res1.results], axis=0)  # [N, KK] fp8

    in_maps2 = []
    for i in core_ids:
        sl = slice(MC * i, MC * (i + 1))
        in_maps2.append(
            {
                "lhsT_c": np.ascontiguousarray(AT[:, sl]),
                "rhs_c": B,
            }
        )

    nc2 = _get("k2", _build_k2)
    res2 = _run_spmd(nc2, in_maps2, core_ids)
    LAST_PERF.append(("launch2", res2.exec_time_ns))

    C = np.concatenate([r["out_c"] for r in res2.results], axis=0)  # [KK, KK] f32

    # ---- binarize + degree-normalize (exact fp32 ops, matches reference) ----
    un_g = (C != 0).astype(np.float32)
    deg = un_g.sum(axis=1, keepdims=True, dtype=np.float32)
    deg = np.where(deg > 0, deg, np.float32(1.0))
    un_g = un_g / deg

    return un_g, new_h, idx


# revision 6
# speedup vs baseline: 1.2389x; 1.2389x over previous
"""Trainium2 Bass kernel for the graph top-k pooling module (nn_Pool).

Math (reference):
    gI = g with diagonal forced to 1            [N, N], 0/1
    G2 = gI @ gI ; U = (G2 != 0)                [N, N]
    score = sum_heads sigmoid(h @ W.T + b)      [N]
    vals, idx = top_k(score, kk)                kk = N/2
    new_h = h[idx, :] * vals[:, None]
    un_g = ((U @ U) != 0)[idx, :][:, idx]       [kk, kk]
    out  = un_g / rowsum(un_g), new_h, idx

Key algebraic reduction: row/col selection commutes with the boolean
matmul, so only
    AT = U[idx, :].T = bin(gI^T @ gI[idx, :].T)     [N, kk]
    B  = U[:, idx]   = bin(gI @ gI[:, idx])          [N, kk]
    C  = bin(AT.T @ B)                               [kk, kk]
are needed: 1.375 TFLOP instead of 2.2 TFLOP.  All matmul operands are
exactly 0/1 so fp8e4m3 inputs with fp32 PSUM accumulation are exact
(counts <= 8192 << 2^24); binarize via min(x, 1).

Only the U entries with row in idx OR col in idx are needed; B's selected
rows duplicate AT (B[idx_p, j] = AT[idx_j, p]), so the device computes just
B_rest = U[rest, idx] -> 1.1 TFLOP total (the entry-wise lower bound).

Distribution (8 cores, 2 launches):
  L1: core i computes AT rows [1024*i : 1024*(i+1)]  (lhsT = gI[:, cols_i],
      rhs = gI[idx,:].T) and B_rest rows [512*i : 512*(i+1)]
      (lhsT = gI.T[:, rest_blk], rhs = gI[:, idx]).  Host gathers AT and
      B_rest and assembles B = U[:, idx].
  L2: core i computes C rows [512*i : 512*(i+1)] = AT[:, slice].T @ B
      as raw fp32 counts.  Host binarizes + row-normalizes (exact fp32 ops).

score/topk/new_h are replicated bit-exactly with the same eager jax ops
as the reference (tiny: 8M FLOP).
"""

import numpy as np

N = 8192
KK = 4096
NCORES = 8
MB = N // NCORES  # 1024: AT/B row-block per core in L1
MC = KK // NCORES  # 512: C row-block per core in L2

_CACHE = {}

# perf results of the last kernel() call: list of (name, exec_time_ns or None)
LAST_PERF = []


def _binarize_evict(nc, psum, sbuf):
    # counts >= 0 are exact integers in PSUM; min(x, 1) -> exact 0/1
    nc.vector.tensor_scalar_min(sbuf, psum, 1.0)


def _build_k1():
    """Launch-1 program per core: AT row-block [8192k x 1024m] @ [8192k x 4096n]
    plus B_rest row-block [8192k x 512m] @ [8192k x 4096n], fp8 in/out,
    binarize-on-evict.

    B's rows at selected indices are recoverable from AT on the host
    (B[idx_p, j] = U[idx_p, idx_j] = AT[idx_j, p]), so the device only
    computes B's rows at non-selected indices."""
    import concourse.mybir as mybir
    import concourse.tile as tile
    from concourse import bacc
    from concourse.kernels.tile_matmul import matmul_tile_kernel

    f8 = mybir.dt.float8e4
    nc = bacc.Bacc("TRN2", target_bir_lowering=False, debug=False)
    lhsT_at = nc.dram_tensor("lhsT_at", [N, MB], f8, kind="ExternalInput")
    rhs_at = nc.dram_tensor("rhs_at", [N, KK], f8, kind="ExternalInput")
    lhsT_b = nc.dram_tensor("lhsT_b", [N, MC], f8, kind="ExternalInput")
    rhs_b = nc.dram_tensor("rhs_b", [N, KK], f8, kind="ExternalInput")
    out_at = nc.dram_tensor("out_at", [MB, KK], f8, kind="ExternalOutput")
    out_b = nc.dram_tensor("out_b", [MC, KK], f8, kind="ExternalOutput")
    with tile.TileContext(nc) as tc:
        matmul_tile_kernel(
            tc, lhsT_at[:], rhs_at[:], out_at[:], psum_evict_fn=_binarize_evict
        )
        matmul_tile_kernel(
            tc, lhsT_b[:], rhs_b[:], out_b[:], psum_evict_fn=_binarize_evict
        )
    nc.compile()
    return nc


def _build_k2():
    """Launch-2 program: [8192k x 512m] @ [8192k x 4096n] fp8 matmul,
    raw fp32 counts out."""
    import concourse.mybir as mybir
    import concourse.tile as tile
    from concourse import bacc
    from concourse.kernels.tile_matmul import matmul_tile_kernel

    f8 = mybir.dt.float8e4
    f32 = mybir.dt.float32
    nc = bacc.Bacc("TRN2", target_bir_lowering=False, debug=False)
    lhsT_c = nc.dram_tensor("lhsT_c", [N, MC], f8, kind="ExternalInput")
    rhs_c = nc.dram_tensor("rhs_c", [N, KK], f8, kind="ExternalInput")
    out_c = nc.dram_tensor("out_c", [MC, KK], f32, kind="ExternalOutput")
    with tile.TileContext(nc) as tc:
        matmul_tile_kernel(tc, lhsT_c[:], rhs_c[:], out_c[:])
    nc.compile()
    return nc


def _get(name, builder):
    if name not in _CACHE:
        _CACHE[name] = builder()
    return _CACHE[name]


def _run_spmd(nc, in_maps, core_ids):
    """run_bass_kernel_spmd, falling back to trace-disabled execution if the
    profiling path is unavailable in this environment."""
    import os
    from concourse.bass_utils import run_bass_kernel_spmd

    try:
        return run_bass_kernel_spmd(nc, in_maps, core_ids)
    except (ImportError, ModuleNotFoundError):
        old = os.environ.get("BASS_NEVER_TRACE")
        os.environ["BASS_NEVER_TRACE"] = "1"
        try:
            return run_bass_kernel_spmd(nc, in_maps, core_ids)
        finally:
            if old is None:
                os.environ.pop("BASS_NEVER_TRACE", None)
            else:
                os.environ["BASS_NEVER_TRACE"] = old


def kernel(g, h, W, b):
    import ml_dtypes
    import jax
    import jax.numpy as jnp

    global LAST_PERF
    LAST_PERF = []

    # ---- score / topk / new_h: bit-exact replication of the reference ----
    h_j = jnp.asarray(h)
    score = jnp.sum(jax.nn.sigmoid(h_j @ jnp.asarray(W).T + jnp.asarray(b)), axis=-1)
    vals, idx_j = jax.lax.top_k(score, KK)
    new_h = np.asarray(h_j[idx_j, :] * vals[:, None])
    idx = np.asarray(idx_j)

    # ---- build 0/1 fp8 operands on host ----
    f8 = ml_dtypes.float8_e4m3
    one8 = np.float32(1.0).astype(f8).view(np.uint8)  # bit pattern of 1.0
    g_np = np.asarray(g)
    gI = np.where(g_np != 0, one8, np.uint8(0))
    np.fill_diagonal(gI, one8)
    gIT = np.ascontiguousarray(gI.T)
    rhs_at = np.ascontiguousarray(gIT[:, idx]).view(f8)  # gI[idx,:].T  [N, KK]
    rhs_b = np.ascontiguousarray(gI[:, idx]).view(f8)  # gI[:, idx]   [N, KK]

    # complement of idx: B rows the device must compute (B[idx,:] comes
    # from AT on the host)
    sel_mask = np.zeros(N, dtype=bool)
    sel_mask[idx] = True
    rest = np.nonzero(~sel_mask)[0]  # sorted, [N - KK]

    core_ids = list(range(NCORES))
    in_maps1 = []
    for i in core_ids:
        sl = slice(MB * i, MB * (i + 1))
        rest_blk = rest[MC * i : MC * (i + 1)]
        in_maps1.append(
            {
                "lhsT_at": np.ascontiguousarray(gI[:, sl]).view(f8),
                "rhs_at": rhs_at,
                "lhsT_b": np.ascontiguousarray(gIT[:, rest_blk]).view(f8),
                "rhs_b": rhs_b,
            }
        )

    nc1 = _get("k1", _build_k1)
    res1 = _run_spmd(nc1, in_maps1, core_ids)
    LAST_PERF.append(("launch1", res1.exec_time_ns))

    AT = np.concatenate([r["out_at"] for r in res1.results], axis=0)  # [N, KK] fp8
    B_rest = np.concatenate([r["out_b"] for r in res1.results], axis=0)  # [N-KK, KK]

    # assemble full B = U[:, idx]: selected rows from AT, the rest from B_rest
    B = np.empty((N, KK), dtype=AT.dtype)
    B[idx] = np.ascontiguousarray(AT.view(np.uint8)[idx].T).view(AT.dtype)
    B[rest] = B_rest

    in_maps2 = []
    for i in core_ids:
        sl = slice(MC * i, MC * (i + 1))
        in_maps2.append(
            {
                "lhsT_c": np.ascontiguousarray(AT[:, sl]),
                "rhs_c": B,
            }
        )

    nc2 = _get("k2", _build_k2)
    res2 = _run_spmd(nc2, in_maps2, core_ids)
    LAST_PERF.append(("launch2", res2.exec_time_ns))

    C = np.concatenate([r["out_c"] for r in res2.results], axis=0)  # [KK, KK] f32

    # ---- binarize + degree-normalize (exact fp32 ops, matches reference) ----
    un_g = (C != 0).astype(np.float32)
    deg = un_g.sum(axis=1, keepdims=True, dtype=np.float32)
    deg = np.where(deg > 0, deg, np.float32(1.0))
    un_g = un_g / deg

    return un_g, new_h, idx


# revision 11
# speedup vs baseline: 6.1216x; 4.9412x over previous
"""Trainium2 Bass kernel for the graph top-k pooling module (nn_Pool).

Math (reference):
    gI = g with diagonal forced to 1            [N, N], 0/1
    U  = ((gI @ gI) != 0)                       [N, N]  (2-hop reachability)
    score = sum_heads sigmoid(h @ W.T + b)      [N]
    vals, idx = top_k(score, kk)                kk = N/2
    new_h = h[idx, :] * vals[:, None]
    C  = ((U @ U) != 0)[idx, :][:, idx]         [kk, kk] (4-hop, selected)
    out = C / rowsum(C), new_h, idx

Exact certification algorithm (validated on the fixed inputs):
    C[i,j] = OR_k U[idx_i, k] & U[k, idx_j].  Restricting the OR to
    k in idx[:KA] (KA=1024 anchors) gives a LOWER bound C1 <= C that is
    exact wherever C1=1.  U restricted to selected rows/cols is ~13%
    dense, so P(C1=0 while C=1) ~ exp(-21): measured 139 uncertified
    entries out of 16.7M.  Those few entries are recomputed exactly on
    the host (full-k contraction via two thin BLAS products).  This is
    exact for ANY input -- an adversarial graph only shifts work to the
    host fallback, never changes results.

    Device work: T1 = U[idx, idx[:KA]]  = bin(gI[idx,:]  @ gI[:,idx[:KA]])
                 T2 = U[idx[:KA], idx]  = bin(gI[idx[:KA],:] @ gI[:,idx])
                 C1 = bin(T1 @ T2)
    = 0.172 TFLOP (vs 2.2 TFLOP naive).  All operands are exactly 0/1 so
    fp8e4m3 with fp32 PSUM accumulation is exact; binarize via min(x,1).

Distribution (8 cores, 2 launches; host reshuffles between launches):
  L1: core i computes T1 rows [512i:512(i+1)] (lhsT = gI.T[:, idx_blk],
      rhs = gI[:, idx[:KA]]) and T2.T rows [512i:512(i+1)]
      (lhsT = gI[:, idx_blk], rhs = gI.T[:, idx[:KA]]).
  L2: core i computes C1 rows [512i:512(i+1)] = T1T[:, blk].T @ T2.

score/topk/new_h are replicated bit-exactly with the same eager jax ops
as the reference (tiny: 8M FLOP).  Final binarize/normalize are exact
fp32 host ops identical to the reference's.
"""

import numpy as np

N = 8192
KK = 4096
KA = 1024  # certification anchors: contraction subset idx[:KA]
NCORES = 8
MC = KK // NCORES  # 512: T1/T2T/C1 row-block per core

_CACHE = {}

# perf results of the last kernel() call: list of (name, exec_time_ns or None)
LAST_PERF = []


def _binarize_evict(nc, psum, sbuf):
    # counts >= 0 are exact integers in PSUM; min(x, 1) -> exact 0/1
    nc.vector.tensor_scalar_min(sbuf, psum, 1.0)


def _build_k1():
    """Launch-1: two [8192k x 512m] @ [8192k x 1024n] fp8 matmuls with
    binarize-on-evict (T1 row-block and T2^T row-block per core)."""
    import concourse.mybir as mybir
    import concourse.tile as tile
    from concourse import bacc
    from concourse.kernels.tile_matmul import matmul_tile_kernel

    f8 = mybir.dt.float8e4
    nc = bacc.Bacc("TRN2", target_bir_lowering=False, debug=False)
    lhsT_a = nc.dram_tensor("lhsT_a", [N, MC], f8, kind="ExternalInput")
    rhs_a = nc.dram_tensor("rhs_a", [N, KA], f8, kind="ExternalInput")
    lhsT_b = nc.dram_tensor("lhsT_b", [N, MC], f8, kind="ExternalInput")
    rhs_b = nc.dram_tensor("rhs_b", [N, KA], f8, kind="ExternalInput")
    out_a = nc.dram_tensor("out_a", [MC, KA], f8, kind="ExternalOutput")
    out_b = nc.dram_tensor("out_b", [MC, KA], f8, kind="ExternalOutput")
    with tile.TileContext(nc) as tc:
        matmul_tile_kernel(
            tc, lhsT_a[:], rhs_a[:], out_a[:], psum_evict_fn=_binarize_evict
        )
        matmul_tile_kernel(
            tc, lhsT_b[:], rhs_b[:], out_b[:], psum_evict_fn=_binarize_evict
        )
    nc.compile()
    return nc


def _build_k2():
    """Launch-2: [1024k x 512m] @ [1024k x 4096n] fp8 matmul, binarized
    fp8 out (C1 row-block per core)."""
    import concourse.mybir as mybir
    import concourse.tile as tile
    from concourse import bacc
    from concourse.kernels.tile_matmul import matmul_tile_kernel

    f8 = mybir.dt.float8e4
    nc = bacc.Bacc("TRN2", target_bir_lowering=False, debug=False)
    lhsT_c = nc.dram_tensor("lhsT_c", [KA, MC], f8, kind="ExternalInput")
    rhs_c = nc.dram_tensor("rhs_c", [KA, KK], f8, kind="ExternalInput")
    out_c = nc.dram_tensor("out_c", [MC, KK], f8, kind="ExternalOutput")
    with tile.TileContext(nc) as tc:
        matmul_tile_kernel(
            tc, lhsT_c[:], rhs_c[:], out_c[:], psum_evict_fn=_binarize_evict
        )
    nc.compile()
    return nc


def _get(name, builder):
    if name not in _CACHE:
        _CACHE[name] = builder()
    return _CACHE[name]


def _run_spmd(nc, in_maps, core_ids):
    """run_bass_kernel_spmd, falling back to trace-disabled execution if the
    profiling path is unavailable in this environment."""
    import os
    from concourse.bass_utils import run_bass_kernel_spmd

    try:
        return run_bass_kernel_spmd(nc, in_maps, core_ids)
    except Exception:
        # e.g. the NTFF profiling hook is unavailable in this environment;
        # retry with tracing hard-disabled.
        old = os.environ.get("BASS_NEVER_TRACE")
        os.environ["BASS_NEVER_TRACE"] = "1"
        try:
            return run_bass_kernel_spmd(nc, in_maps, core_ids)
        finally:
            if old is None:
                os.environ.pop("BASS_NEVER_TRACE", None)
            else:
                os.environ["BASS_NEVER_TRACE"] = old


def kernel(g, h, W, b):
    import ml_dtypes
    import jax
    import jax.numpy as jnp

    global LAST_PERF
    LAST_PERF = []

    # ---- score / topk / new_h: bit-exact replication of the reference ----
    h_j = jnp.asarray(h)
    score = jnp.sum(jax.nn.sigmoid(h_j @ jnp.asarray(W).T + jnp.asarray(b)), axis=-1)
    vals, idx_j = jax.lax.top_k(score, KK)
    new_h = np.asarray(h_j[idx_j, :] * vals[:, None])
    idx = np.asarray(idx_j)
    idxa = idx[:KA]

    # ---- build 0/1 fp8 operands on host ----
    f8 = ml_dtypes.float8_e4m3
    one8 = np.float32(1.0).astype(f8).view(np.uint8)  # bit pattern of 1.0
    g_np = np.asarray(g)
    gI = np.where(g_np != 0, one8, np.uint8(0))
    np.fill_diagonal(gI, one8)
    gIT = np.ascontiguousarray(gI.T)
    rhs_a = np.ascontiguousarray(gI[:, idxa]).view(f8)  # [N, KA]
    rhs_b = np.ascontiguousarray(gIT[:, idxa]).view(f8)  # [N, KA]

    core_ids = list(range(NCORES))
    in_maps1 = []
    for i in core_ids:
        blk = idx[MC * i : MC * (i + 1)]
        in_maps1.append(
            {
                "lhsT_a": np.ascontiguousarray(gIT[:, blk]).view(f8),
                "rhs_a": rhs_a,
                "lhsT_b": np.ascontiguousarray(gI[:, blk]).view(f8),
                "rhs_b": rhs_b,
            }
        )

    nc1 = _get("k1", _build_k1)
    res1 = _run_spmd(nc1, in_maps1, core_ids)
    LAST_PERF.append(("launch1", res1.exec_time_ns))

    T1 = np.concatenate([r["out_a"] for r in res1.results], axis=0)  # [KK, KA] fp8
    T2T = np.concatenate([r["out_b"] for r in res1.results], axis=0)  # [KK, KA] fp8
    T1T = np.ascontiguousarray(T1.view(np.uint8).T)  # [KA, KK] uint8
    T2 = np.ascontiguousarray(T2T.view(np.uint8).T).view(f8)  # [KA, KK] fp8

    in_maps2 = []
    for i in core_ids:
        sl = slice(MC * i, MC * (i + 1))
        in_maps2.append(
            {
                "lhsT_c": np.ascontiguousarray(T1T[:, sl]).view(f8),
                "rhs_c": T2,
            }
        )

    nc2 = _get("k2", _build_k2)
    res2 = _run_spmd(nc2, in_maps2, core_ids)
    LAST_PERF.append(("launch2", res2.exec_time_ns))

    C1 = np.concatenate([r["out_c"] for r in res2.results], axis=0)  # [KK, KK] fp8
    Cb = C1.view(np.uint8) != 0  # bool

    # ---- exact host fallback for entries the anchors did not certify ----
    zer = np.argwhere(~Cb)
    if len(zer):
        gf = (gI != 0).astype(np.float32)
        ui, inv_i = np.unique(zer[:, 0], return_inverse=True)
        uj, inv_j = np.unique(zer[:, 1], return_inverse=True)
        # full-k contraction for just these rows/cols of U (exact fp32)
        Urows = (gf[idx[ui], :] @ gf) != 0  # [Zi, N]
        Ucols = (gf @ gf[:, idx[uj]]) != 0  # [N, Zj]
        hit = (Urows[inv_i] & Ucols[:, inv_j].T).any(axis=1)  # [Z]
        Cb[zer[:, 0], zer[:, 1]] = hit

    # ---- binarize + degree-normalize (exact fp32 ops, matches reference) ----
    un_g = Cb.astype(np.float32)
    deg = un_g.sum(axis=1, keepdims=True, dtype=np.float32)
    deg = np.where(deg > 0, deg, np.float32(1.0))
    un_g = un_g / deg

    return un_g, new_h, idx


# revision 12
# speedup vs baseline: 9.6021x; 1.5686x over previous
"""Trainium2 Bass kernel for the graph top-k pooling module (nn_Pool).

Math (reference):
    gI = g with diagonal forced to 1            [N, N], 0/1
    U  = ((gI @ gI) != 0)                       [N, N]  (2-hop reachability)
    score = sum_heads sigmoid(h @ W.T + b)      [N]
    vals, idx = top_k(score, kk)                kk = N/2
    new_h = h[idx, :] * vals[:, None]
    C  = ((U @ U) != 0)[idx, :][:, idx]         [kk, kk] (4-hop, selected)
    out = C / rowsum(C), new_h, idx

Exact certification algorithm (validated on the fixed inputs):
    C[i,j] = OR_k U[idx_i, k] & U[k, idx_j].  Restricting the OR to
    k in idx[:KA] (KA=1024 anchors) gives a LOWER bound C1 <= C that is
    exact wherever C1=1.  U restricted to selected rows/cols is ~13%
    dense, so P(C1=0 while C=1) ~ exp(-21): measured 139 uncertified
    entries out of 16.7M.  Those few entries are recomputed exactly on
    the host (full-k contraction via two thin BLAS products).  This is
    exact for ANY input -- an adversarial graph only shifts work to the
    host fallback, never changes results.

    Device work: T1 = U[idx, idx[:KA]]  = bin(gI[idx,:]  @ gI[:,idx[:KA]])
                 T2 = U[idx[:KA], idx]  = bin(gI[idx[:KA],:] @ gI[:,idx])
                 C1 = bin(T1 @ T2)
    = 0.172 TFLOP (vs 2.2 TFLOP naive).  All operands are exactly 0/1 so
    fp8e4m3 with fp32 PSUM accumulation is exact; binarize via min(x,1).

Distribution (8 cores, 2 launches; host reshuffles between launches):
  L1: core i computes T1 rows [512i:512(i+1)] (lhsT = gI.T[:, idx_blk],
      rhs = gI[:, idx[:KA]]) and T2.T rows [512i:512(i+1)]
      (lhsT = gI[:, idx_blk], rhs = gI.T[:, idx[:KA]]).
  L2: core i computes C1 rows [512i:512(i+1)] = T1T[:, blk].T @ T2.

score/topk/new_h are replicated bit-exactly with the same eager jax ops
as the reference (tiny: 8M FLOP).  Final binarize/normalize are exact
fp32 host ops identical to the reference's.
"""

import numpy as np

N = 8192
KK = 4096
KA = 512  # certification anchors: contraction subset idx[:KA]
NCORES = 8
MC = KK // NCORES  # 512: T1/T2T/C1 row-block per core

_CACHE = {}

# perf results of the last kernel() call: list of (name, exec_time_ns or None)
LAST_PERF = []


def _binarize_evict(nc, psum, sbuf):
    # counts >= 0 are exact integers in PSUM; min(x, 1) -> exact 0/1
    nc.vector.tensor_scalar_min(sbuf, psum, 1.0)


def _build_k1():
    """Launch-1: two [8192k x 512m] @ [8192k x 1024n] fp8 matmuls with
    binarize-on-evict (T1 row-block and T2^T row-block per core)."""
    import concourse.mybir as mybir
    import concourse.tile as tile
    from concourse import bacc
    from concourse.kernels.tile_matmul import matmul_tile_kernel

    f8 = mybir.dt.float8e4
    nc = bacc.Bacc("TRN2", target_bir_lowering=False, debug=False)
    lhsT_a = nc.dram_tensor("lhsT_a", [N, MC], f8, kind="ExternalInput")
    rhs_a = nc.dram_tensor("rhs_a", [N, KA], f8, kind="ExternalInput")
    lhsT_b = nc.dram_tensor("lhsT_b", [N, MC], f8, kind="ExternalInput")
    rhs_b = nc.dram_tensor("rhs_b", [N, KA], f8, kind="ExternalInput")
    out_a = nc.dram_tensor("out_a", [MC, KA], f8, kind="ExternalOutput")
    out_b = nc.dram_tensor("out_b", [MC, KA], f8, kind="ExternalOutput")
    with tile.TileContext(nc) as tc:
        matmul_tile_kernel(
            tc, lhsT_a[:], rhs_a[:], out_a[:], psum_evict_fn=_binarize_evict
        )
        matmul_tile_kernel(
            tc, lhsT_b[:], rhs_b[:], out_b[:], psum_evict_fn=_binarize_evict
        )
    nc.compile()
    return nc


def _build_k2():
    """Launch-2: [1024k x 512m] @ [1024k x 4096n] fp8 matmul, binarized
    fp8 out (C1 row-block per core)."""
    import concourse.mybir as mybir
    import concourse.tile as tile
    from concourse import bacc
    from concourse.kernels.tile_matmul import matmul_tile_kernel

    f8 = mybir.dt.float8e4
    nc = bacc.Bacc("TRN2", target_bir_lowering=False, debug=False)
    lhsT_c = nc.dram_tensor("lhsT_c", [KA, MC], f8, kind="ExternalInput")
    rhs_c = nc.dram_tensor("rhs_c", [KA, KK], f8, kind="ExternalInput")
    out_c = nc.dram_tensor("out_c", [MC, KK], f8, kind="ExternalOutput")
    with tile.TileContext(nc) as tc:
        matmul_tile_kernel(
            tc, lhsT_c[:], rhs_c[:], out_c[:], psum_evict_fn=_binarize_evict
        )
    nc.compile()
    return nc


def _get(name, builder):
    if name not in _CACHE:
        _CACHE[name] = builder()
    return _CACHE[name]


def _run_spmd(nc, in_maps, core_ids):
    """run_bass_kernel_spmd, falling back to trace-disabled execution if the
    profiling path is unavailable in this environment."""
    import os
    from concourse.bass_utils import run_bass_kernel_spmd

    try:
        return run_bass_kernel_spmd(nc, in_maps, core_ids)
    except Exception:
        # e.g. the NTFF profiling hook is unavailable in this environment;
        # retry with tracing hard-disabled.
        old = os.environ.get("BASS_NEVER_TRACE")
        os.environ["BASS_NEVER_TRACE"] = "1"
        try:
            return run_bass_kernel_spmd(nc, in_maps, core_ids)
        finally:
            if old is None:
                os.environ.pop("BASS_NEVER_TRACE", None)
            else:
                os.environ["BASS_NEVER_TRACE"] = old


def kernel(g, h, W, b):
    import ml_dtypes
    import jax
    import jax.numpy as jnp

    global LAST_PERF
    LAST_PERF = []

    # ---- score / topk / new_h: bit-exact replication of the reference ----
    h_j = jnp.asarray(h)
    score = jnp.sum(jax.nn.sigmoid(h_j @ jnp.asarray(W).T + jnp.asarray(b)), axis=-1)
    vals, idx_j = jax.lax.top_k(score, KK)
    new_h = np.asarray(h_j[idx_j, :] * vals[:, None])
    idx = np.asarray(idx_j)
    idxa = idx[:KA]

    # ---- build 0/1 fp8 operands on host ----
    f8 = ml_dtypes.float8_e4m3
    one8 = np.float32(1.0).astype(f8).view(np.uint8)  # bit pattern of 1.0
    g_np = np.asarray(g)
    gI = np.where(g_np != 0, one8, np.uint8(0))
    np.fill_diagonal(gI, one8)
    gIT = np.ascontiguousarray(gI.T)
    rhs_a = np.ascontiguousarray(gI[:, idxa]).view(f8)  # [N, KA]
    rhs_b = np.ascontiguousarray(gIT[:, idxa]).view(f8)  # [N, KA]

    core_ids = list(range(NCORES))
    in_maps1 = []
    for i in core_ids:
        blk = idx[MC * i : MC * (i + 1)]
        in_maps1.append(
            {
                "lhsT_a": np.ascontiguousarray(gIT[:, blk]).view(f8),
                "rhs_a": rhs_a,
                "lhsT_b": np.ascontiguousarray(gI[:, blk]).view(f8),
                "rhs_b": rhs_b,
            }
        )

    nc1 = _get("k1", _build_k1)
    res1 = _run_spmd(nc1, in_maps1, core_ids)
    LAST_PERF.append(("launch1", res1.exec_time_ns))

    T1 = np.concatenate([r["out_a"] for r in res1.results], axis=0)  # [KK, KA] fp8
    T2T = np.concatenate([r["out_b"] for r in res1.results], axis=0)  # [KK, KA] fp8
    T1T = np.ascontiguousarray(T1.view(np.uint8).T)  # [KA, KK] uint8
    T2 = np.ascontiguousarray(T2T.view(np.uint8).T).view(f8)  # [KA, KK] fp8

    in_maps2 = []
    for i in core_ids:
        sl = slice(MC * i, MC * (i + 1))
        in_maps2.append(
            {
                "lhsT_c": np.ascontiguousarray(T1T[:, sl]).view(f8),
                "rhs_c": T2,
            }
        )

    nc2 = _get("k2", _build_k2)
    res2 = _run_spmd(nc2, in_maps2, core_ids)
    LAST_PERF.append(("launch2", res2.exec_time_ns))

    C1 = np.concatenate([r["out_c"] for r in res2.results], axis=0)  # [KK, KK] fp8
    Cb = C1.view(np.uint8) != 0  # bool

    # ---- exact host fallback for entries the anchors did not certify ----
    zer = np.argwhere(~Cb)
    if len(zer):
        gIb = gI != 0  # bool adjacency (diag set)
        gIbT = gIT != 0
        ui, inv_i = np.unique(zer[:, 0], return_inverse=True)
        uj, inv_j = np.unique(zer[:, 1], return_inverse=True)
        # full-k contraction for just these rows/cols of U, via sparse ORs:
        # U[r, :] = OR of gI rows over out-neighbors of r (incl. diagonal)
        Urows = np.empty((len(ui), N), dtype=bool)
        for n, i_ in enumerate(ui):
            Urows[n] = gIb[gIb[idx[i_]]].any(axis=0)
        Ucols = np.empty((len(uj), N), dtype=bool)
        for n, j_ in enumerate(uj):
            Ucols[n] = gIbT[gIbT[idx[j_]]].any(axis=0)
        hit = np.empty(len(zer), dtype=bool)
        CH = 4096
        for o in range(0, len(zer), CH):
            sl = slice(o, o + CH)
            hit[sl] = (Urows[inv_i[sl]] & Ucols[inv_j[sl]]).any(axis=1)
        Cb[zer[:, 0], zer[:, 1]] = hit

    # ---- binarize + degree-normalize (exact fp32 ops, matches reference) ----
    un_g = Cb.astype(np.float32)
    deg = un_g.sum(axis=1, keepdims=True, dtype=np.float32)
    deg = np.where(deg > 0, deg, np.float32(1.0))
    un_g = un_g / deg

    return un_g, new_h, idx
